# revision 14
# baseline (speedup 1.0000x reference)
"""BasisExpansionLayer Trainium2 kernel.

Full input x: [256, 512] f32. Full output: [256, 512 + 512*512 + 512] f32
laid out as [sin(x) | (x_i * x_j for the cartesian i,j grid) | x].

Sharding: the outer-product index i (512 values) is split across the 8
cores, 64 i-values each.  Every core holds the full batch (256 rows ->
2 x 128 SBUF partitions), so the DVE/ACT engines run with all 128 lanes
busy and both multiply operands are direct slices of the loaded x tiles
(no transposes, no cross-partition broadcasts, no communication).

Per core:
  pair_out[b, i_local*512 + j] = x[b, j] * x[b, c*64 + i_local]
  sin_out[b, i_local]          = sin(x[b, c*64 + i_local])
  id_out[b, i_local]           = x[b, c*64 + i_local]
The host reassembles the full [256, 263168] output from the 8 shards.
"""

import numpy as np

B = 256
D = 512
M = 8            # cores
IPC = D // M     # 64 i-values per core

_CACHE = {}

# sin(x) = y * p(y^2), y = x - round(x/2pi)*2pi (Cody-Waite), |y| <= pi.
# p coeffs: IRLS-minimax fit, end-to-end fp32 max abs err 5.3e-7.
SIN_COEFFS = [
    1.0,
    -0.166666641831398,
    0.00833331048488617,
    -0.0001984015543712303,
    2.752945647443994e-06,
    -2.467699466990325e-08,
    1.345159122978501e-10,
]
INV2PI = 0.15915494309189535
MAGIC = 12582912.0  # 1.5 * 2**23: fp32 round-to-nearest via add/sub
TWOPI_HI = 6.28125
TWOPI_LO = 0.0019353071795864769

# knobs: tile_plan = per-half list of store-tile sizes (i-values per DMA),
# act_every = every act_every-th multiply goes to the scalar engine.
DEFAULT_CFG = dict(
    tile_plan=(2, 4, 8, 12, 16, 16, 6),
    act_every=4,
    bufs=5,
    repeat=1,
    sin_late=True,   # emit the sin chain after the first pair tile
)


def _build_nc(cfg=None):
    import concourse.bass as bass  # noqa: F401
    import concourse.mybir as mybir
    import concourse.tile as tile
    from concourse import bacc

    cfg = {**DEFAULT_CFG, **(cfg or {})}
    tile_plan = cfg["tile_plan"]
    act_every = cfg["act_every"]
    bufs = cfg["bufs"]
    repeat = cfg["repeat"]
    sin_late = cfg["sin_late"]
    assert sum(tile_plan) == IPC, tile_plan

    f32 = mybir.dt.float32
    nc = bacc.Bacc("TRN2", target_bir_lowering=False, debug=False, num_devices=M)

    x = nc.dram_tensor("x", [B, D], f32, kind="ExternalInput")
    xs = nc.dram_tensor("xs", [B, IPC], f32, kind="ExternalInput")
    sin_out = nc.dram_tensor("sin_out", [B, IPC], f32, kind="ExternalOutput")
    pair_out = nc.dram_tensor("pair_out", [B, IPC * D], f32, kind="ExternalOutput")
    id_out = nc.dram_tensor("id_out", [B, IPC], f32, kind="ExternalOutput")

    with tile.TileContext(nc) as tc:
        with (
            tc.tile_pool(name="xp", bufs=1) as xpool,
            tc.tile_pool(name="sp", bufs=2) as spool,
            tc.tile_pool(name="op", bufs=bufs) as opool,
        ):
            alu = mybir.AluOpType
            for _rep in range(repeat):
                xt = []
                xst = []
                for h in range(2):
                    rows = slice(h * 128, (h + 1) * 128)
                    ts = xpool.tile([128, IPC], f32, tag=f"xs{h}")
                    nc.sync.dma_start(ts[:], xs[rows, :])
                    xst.append(ts)
                    t = xpool.tile([128, D], f32, tag=f"x{h}")
                    nc.sync.dma_start(t[:], x[rows, :])
                    xt.append(t)

                # identity (tiny): straight DMA of the xs tiles.
                for h in range(2):
                    rows = slice(h * 128, (h + 1) * 128)
                    nc.sync.dma_start(id_out[rows, :], xst[h][:])

                def emit_sin():
                    # accurate sin via odd polynomial on DVE: both batch
                    # halves side by side in one [128, 2*IPC] tile.
                    W = 2 * IPC
                    xsin = spool.tile([128, W], f32, tag="xsin")
                    for h in range(2):
                        nc.sync.dma_start(
                            xsin[:, h * IPC : (h + 1) * IPC],
                            xs[h * 128 : (h + 1) * 128, :],
                        )
                    tt = spool.tile([128, W], f32, tag="t")
                    # t = x*inv2pi + magic ; k = t - magic (round-to-nearest)
                    nc.vector.tensor_scalar(
                        out=tt[:], in0=xsin[:], scalar1=INV2PI, scalar2=MAGIC,
                        op0=alu.mult, op1=alu.add,
                    )
                    kk = spool.tile([128, W], f32, tag="k")
                    nc.vector.tensor_scalar_sub(kk[:], tt[:], MAGIC)
                    # y = (x - k*2pi_hi) - k*2pi_lo
                    kh = spool.tile([128, W], f32, tag="kh")
                    nc.vector.tensor_scalar_mul(kh[:], kk[:], TWOPI_HI)
                    yy = spool.tile([128, W], f32, tag="y")
                    nc.vector.tensor_sub(yy[:], xsin[:], kh[:])
                    kl = spool.tile([128, W], f32, tag="kl")
                    nc.vector.tensor_scalar_mul(kl[:], kk[:], TWOPI_LO)
                    nc.vector.tensor_sub(yy[:], yy[:], kl[:])
                    uu = spool.tile([128, W], f32, tag="u")
                    nc.vector.tensor_mul(uu[:], yy[:], yy[:])
                    # Horner: p = (((c6*u + c5)*u + c4)...)*u + c0
                    pp = spool.tile([128, W], f32, tag="p")
                    nc.vector.tensor_scalar(
                        out=pp[:], in0=uu[:], scalar1=SIN_COEFFS[6],
                        scalar2=SIN_COEFFS[5], op0=alu.mult, op1=alu.add,
                    )
                    for cidx in (4, 3, 2, 1, 0):
                        nc.vector.tensor_mul(pp[:], pp[:], uu[:])
                        nc.vector.tensor_scalar_add(
                            pp[:], pp[:], SIN_COEFFS[cidx]
                        )
                    ss = spool.tile([128, W], f32, tag="s")
                    nc.vector.tensor_mul(ss[:], pp[:], yy[:])
                    for h in range(2):
                        rows = slice(h * 128, (h + 1) * 128)
                        nc.sync.dma_start(
                            sin_out[rows, :], ss[:, h * IPC : (h + 1) * IPC]
                        )

                if not sin_late:
                    emit_sin()

                # pair part: out[p, k*512 + j] = x[p, j] * x[p, i]
                n_op = 0
                n_tile = 0
                for h in range(2):
                    rows = slice(h * 128, (h + 1) * 128)
                    i0 = 0
                    for g_sz in tile_plan:
                        ot = opool.tile([128, g_sz * D], f32, tag="out")
                        for k in range(g_sz):
                            i = i0 + k
                            dst = ot[:, k * D : (k + 1) * D]
                            scal = xst[h][:, i : i + 1]
                            if n_op % act_every == act_every - 1:
                                # ACT: out = in * scale (activation Copy)
                                nc.scalar.mul(dst, xt[h][:], scal)
                            else:
                                nc.vector.tensor_scalar_mul(dst, xt[h][:], scal)
                            n_op += 1
                        nc.sync.dma_start(
                            pair_out[rows, i0 * D : (i0 + g_sz) * D], ot[:]
                        )
                        i0 += g_sz
                        n_tile += 1
                        if sin_late and n_tile == 1:
                            emit_sin()
    nc.compile()
    return nc


def _get_nc(cfg=None):
    key = repr(cfg)
    if key not in _CACHE:
        _CACHE[key] = _build_nc(cfg)
    return _CACHE[key]


def _in_maps(x):
    x = np.ascontiguousarray(np.asarray(x, dtype=np.float32))
    assert x.shape == (B, D)
    return [
        {
            "x": x,
            "xs": np.ascontiguousarray(x[:, c * IPC : (c + 1) * IPC]),
        }
        for c in range(M)
    ]


def _get_exec(cfg=None):
    """Build the 8-core sharded PJRT callable once per process.

    Mirrors bass2jax.run_bass_via_pjrt's multi-core path, but caches the
    jitted executable: loading/executing a second NEFF in the same process
    can wedge the exec unit, while re-executing one cached executable with
    donated output buffers is reliable.
    """
    key = ("exec", repr(cfg))
    if key in _CACHE:
        return _CACHE[key]

    import jax
    from jax.sharding import Mesh, PartitionSpec
    from jax.experimental.shard_map import shard_map
    import concourse.mybir as mybir
    from concourse import bass2jax

    nc = _get_nc(cfg)
    bass2jax.install_neuronx_cc_hook()

    partition_name = nc.partition_id_tensor.name if nc.partition_id_tensor else None
    in_names, out_names, out_avals, out_shapes = [], [], [], []
    for alloc in nc.m.functions[0].allocations:
        if not isinstance(alloc, mybir.MemoryLocationSet):
            continue
        name = alloc.memorylocations[0].name
        if alloc.kind == "ExternalInput":
            if name != partition_name:
                in_names.append(name)
        elif alloc.kind == "ExternalOutput":
            shape = tuple(alloc.tensor_shape)
            dtype = mybir.dt.np(alloc.dtype)
            out_names.append(name)
            out_avals.append(jax.core.ShapedArray(shape, dtype))
            out_shapes.append((shape, dtype))
    n_params = len(in_names)
    n_outs = len(out_avals)
    all_in_names = list(in_names) + list(out_names)
    if partition_name is not None:
        all_in_names.append(partition_name)

    def _body(*args):
        operands = list(args)
        if partition_name is not None:
            operands.append(bass2jax.partition_id_tensor())
        return tuple(
            bass2jax._bass_exec_p.bind(
                *operands,
                out_avals=tuple(out_avals),
                in_names=tuple(all_in_names),
                out_names=tuple(out_names),
                lowering_input_output_aliases=(),
                sim_require_finite=True,
                sim_require_nnan=True,
                nc=nc,
            )
        )

    devices = jax.devices()[:M]
    assert len(devices) == M, f"need {M} NeuronCores, found {len(devices)}"
    mesh = Mesh(np.asarray(devices), ("core",))
    in_specs = (PartitionSpec("core"),) * (n_params + n_outs)
    out_specs = (PartitionSpec("core"),) * n_outs
    donate = tuple(range(n_params, n_params + n_outs))
    sharded = jax.jit(
        shard_map(_body, mesh=mesh, in_specs=in_specs, out_specs=out_specs,
                  check_rep=False),
        donate_argnums=donate,
        keep_unused=True,
    )

    def run(in_maps):
        concat_in = [
            np.concatenate([np.asarray(in_maps[c][n]) for c in range(M)], axis=0)
            for n in in_names
        ]
        concat_zeros = [
            np.zeros((M * s[0], *s[1:]), dt) for s, dt in out_shapes
        ]
        outs = sharded(*concat_in, *concat_zeros)
        return [
            {
                name: np.asarray(outs[i]).reshape(M, *out_shapes[i][0])[c]
                for i, name in enumerate(out_names)
            }
            for c in range(M)
        ]

    _CACHE[key] = run
    return run


def _run(x, cfg=None):
    from concourse._compat import axon_active

    if axon_active():
        return _get_exec(cfg)(_in_maps(x))
    # native NRT path (no axon): run_bass_kernel_spmd handles the NEFF
    # load/exec/unload lifecycle per call.
    from concourse import bass_utils

    res = bass_utils.run_bass_kernel_spmd(
        _get_nc(cfg), _in_maps(x), core_ids=list(range(M))
    )
    return res.results


def kernel(**inputs):
    results = _run(inputs["x"])
    out = np.empty((B, 2 * D + D * D), dtype=np.float32)
    for c in range(M):
        r = results[c]
        out[:, c * IPC : (c + 1) * IPC] = r["sin_out"]
        out[:, D + c * IPC * D : D + (c + 1) * IPC * D] = r["pair_out"]
        out[:, D + D * D + c * IPC : D + D * D + (c + 1) * IPC] = r["id_out"]
    return out
